# revision 2
# baseline (speedup 1.0000x reference)
"""Bidirectional LSTM layer (T=1024, B=64, I=H=512) on 8 Trainium2 NeuronCores.

Sharding: cores 0-3 run the forward direction, cores 4-7 the backward
direction; each core owns 16 batch columns and runs the full sequential scan.
The length-dependent flip (flip_batch) is done on-device with indirect DMA
gather/scatter driven by small host-computed index tables.
"""

import sys
from contextlib import ExitStack

import numpy as np

for p in ("/opt/trn_rl_repo", "/root/.axon_site/_ro/trn_rl_repo"):
    if p not in sys.path:
        sys.path.append(p)

import concourse.bass as bass
import concourse.tile as tile
from concourse import bacc, mybir
from concourse.bass_utils import run_bass_kernel_spmd
from concourse.masks import make_identity

F32 = mybir.dt.float32
I32 = mybir.dt.int32

T, B, I, H = 1024, 64, 512, 512
G = 4 * H            # gate width
NCORES = 8
BL = B // (NCORES // 2)   # 16 batch columns per core
CT = 16                   # timesteps per pipeline chunk
RPB = 128                 # rows per projection block
SPB = RPB // BL           # steps per projection block (8)
BPC = CT // SPB           # projection blocks per chunk (2)
NK_I = I // 128           # k-tiles for input dim (4)
NK_H = H // 128           # k-tiles for hidden dim (4)
NN = G // 512             # 512-wide gate column chunks (4)

# gate column order in the reordered weights: (f, g, i, o)
# torch row order is (i, f, g, o)
_PERM = np.concatenate([
    np.arange(H, 2 * H),      # f
    np.arange(2 * H, 3 * H),  # g
    np.arange(0, H),          # i
    np.arange(3 * H, 4 * H),  # o
])
BLK_F, BLK_G, BLK_I, BLK_O = 0, 1, 2, 3


def build_program(t_steps=T):
    nsteps = t_steps
    nchunks = nsteps // CT
    nc = bacc.Bacc("TRN2", target_bir_lowering=False, debug=False)

    # DRAM I/O (per-core values supplied via in_maps)
    x_d = nc.dram_tensor("x", [nsteps * BL, I], F32, kind="ExternalInput").ap()
    wih_d = nc.dram_tensor("wihT", [I, G], F32, kind="ExternalInput").ap()
    whh_d = nc.dram_tensor("whhT", [H, G], F32, kind="ExternalInput").ap()
    bias_d = nc.dram_tensor("bias", [1, G], F32, kind="ExternalInput").ap()
    h0T_d = nc.dram_tensor("h0T", [H, BL], F32, kind="ExternalInput").ap()
    c0_d = nc.dram_tensor("c0", [BL, H], F32, kind="ExternalInput").ap()
    # gather offsets: [128, nblocks] column b = row offsets of projection block b
    goff_d = nc.dram_tensor(
        "goff", [RPB, nsteps * BL // RPB], I32, kind="ExternalInput"
    ).ap()
    # scatter offsets: [BL, nsteps]
    soff_d = nc.dram_tensor("soff", [BL, nsteps], I32, kind="ExternalInput").ap()
    out_d = nc.dram_tensor("out", [nsteps * BL, H], F32, kind="ExternalOutput").ap()

    with tile.TileContext(nc) as tc, ExitStack() as ctx:
        cpool = ctx.enter_context(tc.tile_pool(name="consts", bufs=1))
        ident = cpool.tile([128, 128], F32, tag="ident")
        make_identity(nc, ident[:])
        ones_r = cpool.tile([1, 128], F32, tag="ones")
        nc.vector.memset(ones_r[:], 1.0)

        wih_sb = cpool.tile([128, NK_I * G], F32, tag="wih")
        whh_sb = cpool.tile([128, NK_H * G], F32, tag="whh")
        for k in range(NK_I):
            nc.sync.dma_start(
                wih_sb[:, k * G:(k + 1) * G], wih_d[k * 128:(k + 1) * 128, :]
            )
        for k in range(NK_H):
            nc.sync.dma_start(
                whh_sb[:, k * G:(k + 1) * G], whh_d[k * 128:(k + 1) * 128, :]
            )
        bias_sb = cpool.tile([1, G], F32, tag="bias")
        nc.sync.dma_start(bias_sb[:], bias_d[:])
        goff_sb = cpool.tile([RPB, nsteps * BL // RPB], I32, tag="goff")
        nc.sync.dma_start(goff_sb[:], goff_d[:])
        soff_sb = cpool.tile([BL, nsteps], I32, tag="soff")
        nc.sync.dma_start(soff_sb[:], soff_d[:])

        # live state
        hT = cpool.tile([128, NK_H * BL], F32, tag="hT")      # h^T k-tiles
        c_st = cpool.tile([BL, H], F32, tag="c")
        for k in range(NK_H):
            nc.sync.dma_start(
                hT[:, k * BL:(k + 1) * BL], h0T_d[k * 128:(k + 1) * 128, :]
            )
        nc.sync.dma_start(c_st[:], c0_d[:])

        # pipeline pools
        xg_pool = ctx.enter_context(tc.tile_pool(name="xg", bufs=2 * BPC))
        xT_pool = ctx.enter_context(tc.tile_pool(name="xT", bufs=2))
        xp_pool = ctx.enter_context(tc.tile_pool(name="xproj", bufs=2 * BPC))
        act_pool = ctx.enter_context(tc.tile_pool(name="acts", bufs=2))
        tmp_pool = ctx.enter_context(tc.tile_pool(name="tmps", bufs=2))
        hr_pool = ctx.enter_context(tc.tile_pool(name="hring", bufs=4))
        ps_rec_pool = ctx.enter_context(
            tc.tile_pool(name="psrec", bufs=1, space="PSUM")
        )
        ps_bg_pool = ctx.enter_context(
            tc.tile_pool(name="psbg", bufs=1, space="PSUM")
        )
        ps_rec = ps_rec_pool.tile([128, 512 * NN], F32, tag="psrec")
        ps_bg = ps_bg_pool.tile([128, 512 * NN], F32, tag="psbg")

        def background(c):
            """Gather + transpose + project chunk c; returns xproj block tiles."""
            xp_tiles = []
            for b in range(BPC):
                blk = c * BPC + b
                xg = xg_pool.tile([RPB, I], F32, tag="xg")
                nc.gpsimd.indirect_dma_start(
                    out=xg[:],
                    out_offset=None,
                    in_=x_d[:],
                    in_offset=bass.IndirectOffsetOnAxis(
                        ap=goff_sb[:, blk:blk + 1], axis=0
                    ),
                )
                # transpose rows->features: xg [128r, 512i] -> xT 4x[128i, 128r]
                xT = xT_pool.tile([128, I], F32, tag="xT")
                for k in range(NK_I):
                    nc.tensor.transpose(
                        out=ps_bg[:, k * 128:(k + 1) * 128],
                        in_=xg[:, k * 128:(k + 1) * 128],
                        identity=ident[:],
                    )
                nc.vector.tensor_copy(xT[:], ps_bg[:, :I])
                # projection GEMM: xproj[128r, G] = xT.T @ wihT + bias
                xp = xp_pool.tile([RPB, G], F32, tag="xp")
                for n in range(NN):
                    ncols = slice(n * 512, (n + 1) * 512)
                    for k in range(NK_I):
                        nc.tensor.matmul(
                            out=ps_bg[:, ncols],
                            lhsT=xT[:, k * 128:(k + 1) * 128],
                            rhs=wih_sb[:, k * G + n * 512:k * G + (n + 1) * 512],
                            start=(k == 0),
                            stop=False,
                        )
                    nc.tensor.matmul(
                        out=ps_bg[:, ncols],
                        lhsT=ones_r[:1, :RPB],
                        rhs=bias_sb[:1, ncols],
                        start=False,
                        stop=True,
                    )
                nc.vector.tensor_copy(xp[:, : G // 2], ps_bg[:, : G // 2])
                nc.scalar.copy(xp[:, G // 2:], ps_bg[:, G // 2:])
                xp_tiles.append(xp)
            return xp_tiles

        def step(s, xp):
            """One LSTM timestep. xp: xproj block tile holding this step."""
            k_in_blk = (s % CT) % SPB
            sel = ident[:, k_in_blk * BL:(k_in_blk + 1) * BL]
            # gates = xproj_rows + h @ whhT  (into ps_rec[0:BL, :])
            for n in range(NN):
                ncols = slice(n * 512, (n + 1) * 512)
                nc.tensor.matmul(
                    out=ps_rec[:BL, ncols],
                    lhsT=sel,
                    rhs=xp[:, ncols],
                    start=True,
                    stop=False,
                )
                for k in range(NK_H):
                    nc.tensor.matmul(
                        out=ps_rec[:BL, ncols],
                        lhsT=hT[:, k * BL:(k + 1) * BL],
                        rhs=whh_sb[:, k * G + n * 512:k * G + (n + 1) * 512],
                        start=False,
                        stop=(k == NK_H - 1),
                    )
            AF = mybir.ActivationFunctionType
            f_h = act_pool.tile([BL, 512], F32, tag="fh")
            g_h = act_pool.tile([BL, 512], F32, tag="gh")
            i_h = act_pool.tile([BL, 512], F32, tag="ih")
            o_h = act_pool.tile([BL, 512], F32, tag="oh")
            nc.scalar.activation(f_h[:], ps_rec[:BL, BLK_F * 512:(BLK_F + 1) * 512], AF.Sigmoid)
            nc.scalar.activation(g_h[:], ps_rec[:BL, BLK_G * 512:(BLK_G + 1) * 512], AF.Tanh)
            nc.scalar.activation(i_h[:], ps_rec[:BL, BLK_I * 512:(BLK_I + 1) * 512], AF.Sigmoid)
            nc.scalar.activation(o_h[:], ps_rec[:BL, BLK_O * 512:(BLK_O + 1) * 512], AF.Sigmoid)

            m1 = tmp_pool.tile([BL, 512], F32, tag="m1")
            m2 = tmp_pool.tile([BL, 512], F32, tag="m2")
            nc.vector.tensor_mul(m1[:], f_h[:], c_st[:])
            nc.vector.tensor_mul(m2[:], i_h[:], g_h[:])
            nc.vector.tensor_add(c_st[:], m1[:], m2[:])
            tc_h = tmp_pool.tile([BL, 512], F32, tag="tc")
            nc.scalar.activation(tc_h[:], c_st[:], AF.Tanh)
            h_new = hr_pool.tile([BL, 512], F32, tag="h")
            nc.vector.tensor_mul(h_new[:], o_h[:], tc_h[:])

            # h^T for next step's stationary: 4 PE transposes into ps_rec[:, :64]
            for k in range(NK_H):
                nc.tensor.transpose(
                    out=ps_rec[:, k * BL:(k + 1) * BL],
                    in_=h_new[:, k * 128:(k + 1) * 128],
                    identity=ident[:BL, :BL],
                )
            nc.vector.tensor_copy(hT[:], ps_rec[:, :NK_H * BL])

            # scatter this step's output rows
            nc.gpsimd.indirect_dma_start(
                out=out_d[:],
                out_offset=bass.IndirectOffsetOnAxis(
                    ap=soff_sb[:, s:s + 1], axis=0
                ),
                in_=h_new[:],
                in_offset=None,
            )

        xp_cur = background(0)
        for c in range(nchunks):
            if c + 1 < nchunks:
                xp_next = background(c + 1)
            else:
                xp_next = None
            for l in range(CT):
                s = c * CT + l
                step(s, xp_cur[l // SPB])
            xp_cur = xp_next

    nc.compile()
    return nc


def make_core_inputs(inputs, core, t_steps=T):
    """Build the in_map for one core from the full problem inputs."""
    fwd = core < NCORES // 2
    sl = slice((core % 4) * BL, (core % 4) * BL + BL)
    x = np.asarray(inputs["input"], np.float32)[:t_steps, sl, :]
    lengths = np.asarray(inputs["lengths"]).astype(np.int64)[sl]
    sfx = "f" if fwd else "b"
    w_ih = np.asarray(inputs[f"w_ih_{sfx}"], np.float32)
    w_hh = np.asarray(inputs[f"w_hh_{sfx}"], np.float32)
    bias = (
        np.asarray(inputs[f"b_ih_{sfx}"], np.float32)
        + np.asarray(inputs[f"b_hh_{sfx}"], np.float32)
    )
    h0 = np.asarray(inputs[f"h0_{sfx}"], np.float32)[sl]
    c0 = np.asarray(inputs[f"c0_{sfx}"], np.float32)[sl]

    t = np.arange(t_steps, dtype=np.int64)[:, None]
    if fwd:
        idx = np.broadcast_to(t, (t_steps, BL))
    else:
        L = lengths[None, :]
        idx = np.where(t < L, L - 1 - t, t)
    offs = (idx * BL + np.arange(BL)[None, :]).astype(np.int32)  # [T, BL]
    goff = offs.reshape(-1, RPB).T.copy()       # [128, nblocks]
    soff = offs.T.copy()                        # [BL, T]

    return {
        "x": np.ascontiguousarray(x.reshape(t_steps * BL, I)),
        "wihT": np.ascontiguousarray(w_ih.T[:, _PERM]),
        "whhT": np.ascontiguousarray(w_hh.T[:, _PERM]),
        "bias": np.ascontiguousarray(bias[_PERM][None, :]),
        "h0T": np.ascontiguousarray(h0.T),
        "c0": np.ascontiguousarray(c0),
        "goff": np.ascontiguousarray(goff),
        "soff": np.ascontiguousarray(soff),
    }


_PROGRAM_CACHE = {}


def kernel(**inputs) -> np.ndarray:
    t_steps = inputs["input"].shape[0]
    if t_steps not in _PROGRAM_CACHE:
        _PROGRAM_CACHE[t_steps] = build_program(t_steps)
    nc = _PROGRAM_CACHE[t_steps]
    in_maps = [make_core_inputs(inputs, c, t_steps) for c in range(NCORES)]
    res = run_bass_kernel_spmd(nc, in_maps, list(range(NCORES)))
    out = np.empty((t_steps, B, 2 * H), np.float32)
    for c in range(NCORES):
        sl = slice((c % 4) * BL, (c % 4) * BL + BL)
        half = slice(0, H) if c < 4 else slice(H, 2 * H)
        out[:, sl, half] = res.results[c]["out"].reshape(t_steps, BL, H)
    return out


if __name__ == "__main__":
    pass


# revision 4
# speedup vs baseline: 1.5403x; 1.5403x over previous
"""Bidirectional LSTM layer (T=1024, B=64, I=H=512) on 8 Trainium2 NeuronCores.

Sharding: cores 0-3 run the forward direction, cores 4-7 the backward
direction; each core owns 16 batch columns and runs the full sequential scan.
The length-dependent flip (flip_batch) is done on-device with indirect DMA
gather/scatter driven by small host-computed index tables.
"""

import sys
from contextlib import ExitStack

import numpy as np

for p in ("/opt/trn_rl_repo", "/root/.axon_site/_ro/trn_rl_repo"):
    if p not in sys.path:
        sys.path.append(p)

import concourse.bass as bass
import concourse.tile as tile
from concourse import bacc, mybir
from concourse.bass_utils import run_bass_kernel_spmd
from concourse.masks import make_identity

F32 = mybir.dt.float32
F32R = mybir.dt.float32r
I32 = mybir.dt.int32


def r32(ap):
    return ap.bitcast(F32R)

T, B, I, H = 1024, 64, 512, 512
G = 4 * H            # gate width
NCORES = 8
BL = B // (NCORES // 2)   # 16 batch columns per core
CT = 16                   # timesteps per pipeline chunk
RPB = 128                 # rows per projection block
SPB = RPB // BL           # steps per projection block (8)
BPC = CT // SPB           # projection blocks per chunk (2)
NK_I = I // 128           # k-tiles for input dim (4)
NK_H = H // 128           # k-tiles for hidden dim (4)
NN = G // 512             # 512-wide gate column chunks (4)

# gate column order in the reordered weights: (f, g, i, o)
# torch row order is (i, f, g, o)
_PERM = np.concatenate([
    np.arange(H, 2 * H),      # f
    np.arange(2 * H, 3 * H),  # g
    np.arange(0, H),          # i
    np.arange(3 * H, 4 * H),  # o
])
BLK_F, BLK_G, BLK_I, BLK_O = 0, 1, 2, 3


def build_program(t_steps=T):
    nsteps = t_steps
    nchunks = nsteps // CT
    nc = bacc.Bacc("TRN2", target_bir_lowering=False, debug=False)

    # DRAM I/O (per-core values supplied via in_maps)
    x_d = nc.dram_tensor("x", [nsteps * BL, I], F32, kind="ExternalInput").ap()
    wih_d = nc.dram_tensor("wihT", [I, G], F32R, kind="ExternalInput").ap()
    whh_d = nc.dram_tensor("whhT", [H, G], F32R, kind="ExternalInput").ap()
    bias_d = nc.dram_tensor("bias", [1, G], F32R, kind="ExternalInput").ap()
    h0T_d = nc.dram_tensor("h0T", [H, BL], F32R, kind="ExternalInput").ap()
    c0_d = nc.dram_tensor("c0", [BL, H], F32, kind="ExternalInput").ap()
    # gather offsets: [128, nblocks] column b = row offsets of projection block b
    goff_d = nc.dram_tensor(
        "goff", [RPB, nsteps * BL // RPB], I32, kind="ExternalInput"
    ).ap()
    # scatter offsets: [BL, nsteps]
    soff_d = nc.dram_tensor("soff", [BL, nsteps], I32, kind="ExternalInput").ap()
    out_d = nc.dram_tensor("out", [nsteps * BL, H], F32, kind="ExternalOutput").ap()

    with tile.TileContext(nc) as tc, ExitStack() as ctx:
        cpool = ctx.enter_context(tc.tile_pool(name="consts", bufs=1))
        ident = cpool.tile([128, 128], F32, tag="ident")
        make_identity(nc, ident[:])
        identr = cpool.tile([128, 128], F32R, tag="identr")
        nc.vector.tensor_copy(identr[:], ident[:])
        ones_f = cpool.tile([1, 128], F32, tag="onesf")
        nc.vector.memset(ones_f[:], 1.0)
        ones_r = cpool.tile([1, 128], F32R, tag="ones")
        nc.vector.tensor_copy(ones_r[:], ones_f[:])

        wih_sb = cpool.tile([128, NK_I * G], F32R, tag="wih")
        whh_sb = cpool.tile([128, NK_H * G], F32R, tag="whh")
        for k in range(NK_I):
            nc.sync.dma_start(
                wih_sb[:, k * G:(k + 1) * G], wih_d[k * 128:(k + 1) * 128, :]
            )
        for k in range(NK_H):
            nc.sync.dma_start(
                whh_sb[:, k * G:(k + 1) * G], whh_d[k * 128:(k + 1) * 128, :]
            )
        bias_sb = cpool.tile([1, G], F32R, tag="bias")
        nc.sync.dma_start(bias_sb[:], bias_d[:])
        goff_sb = cpool.tile([RPB, nsteps * BL // RPB], I32, tag="goff")
        nc.sync.dma_start(goff_sb[:], goff_d[:])
        soff_sb = cpool.tile([BL, nsteps], I32, tag="soff")
        nc.sync.dma_start(soff_sb[:], soff_d[:])

        # live state
        hT = cpool.tile([128, NK_H * BL], F32R, tag="hT")      # h^T k-tiles
        c_st = cpool.tile([BL, H], F32, tag="c")
        for k in range(NK_H):
            nc.sync.dma_start(
                hT[:, k * BL:(k + 1) * BL], h0T_d[k * 128:(k + 1) * 128, :]
            )
        nc.sync.dma_start(c_st[:], c0_d[:])

        # pipeline pools
        xg_pool = ctx.enter_context(tc.tile_pool(name="xg", bufs=2 * BPC))
        xT_pool = ctx.enter_context(tc.tile_pool(name="xT", bufs=2))
        xp_pool = ctx.enter_context(tc.tile_pool(name="xproj", bufs=2 * BPC))
        act_pool = ctx.enter_context(tc.tile_pool(name="acts", bufs=2))
        tmp_pool = ctx.enter_context(tc.tile_pool(name="tmps", bufs=2))
        hr_pool = ctx.enter_context(tc.tile_pool(name="hring", bufs=4))
        ps_rec_pool = ctx.enter_context(
            tc.tile_pool(name="psrec", bufs=1, space="PSUM")
        )
        ps_bg_pool = ctx.enter_context(
            tc.tile_pool(name="psbg", bufs=1, space="PSUM")
        )
        ps_rec = ps_rec_pool.tile([128, 512 * NN], F32, tag="psrec")
        ps_bg = ps_bg_pool.tile([128, 512 * NN], F32, tag="psbg")

        def background(c):
            """Gather + transpose + project chunk c; returns xproj block tiles."""
            xp_tiles = []
            for b in range(BPC):
                blk = c * BPC + b
                xg = xg_pool.tile([RPB, I], F32, tag="xg")
                nc.gpsimd.indirect_dma_start(
                    out=xg[:],
                    out_offset=None,
                    in_=x_d[:],
                    in_offset=bass.IndirectOffsetOnAxis(
                        ap=goff_sb[:, blk:blk + 1], axis=0
                    ),
                )
                # transpose rows->features: xg [128r, 512i] -> xT 4x[128i, 128r]
                xT = xT_pool.tile([128, I], F32R, tag="xT")
                for k in range(NK_I):
                    nc.tensor.transpose(
                        out=ps_bg[:, k * 128:(k + 1) * 128],
                        in_=xg[:, k * 128:(k + 1) * 128],
                        identity=ident[:],
                    )
                nc.vector.tensor_copy(xT[:], ps_bg[:, :I])
                # projection GEMM: xproj[128r, G] = xT.T @ wihT + bias
                xp = xp_pool.tile([RPB, G], F32R, tag="xp")
                for n in range(NN):
                    ncols = slice(n * 512, (n + 1) * 512)
                    for k in range(NK_I):
                        nc.tensor.matmul(
                            out=ps_bg[:, ncols],
                            lhsT=xT[:, k * 128:(k + 1) * 128],
                            rhs=wih_sb[:, k * G + n * 512:k * G + (n + 1) * 512],
                            start=(k == 0),
                            stop=False,
                        )
                    nc.tensor.matmul(
                        out=ps_bg[:, ncols],
                        lhsT=ones_r[:1, :RPB],
                        rhs=bias_sb[:1, ncols],
                        start=False,
                        stop=True,
                    )
                nc.vector.tensor_copy(xp[:, : G // 2], ps_bg[:, : G // 2])
                nc.scalar.copy(xp[:, G // 2:], ps_bg[:, G // 2:])
                xp_tiles.append(xp)
            return xp_tiles

        def step(s, xp):
            """One LSTM timestep. xp: xproj block tile holding this step."""
            k_in_blk = (s % CT) % SPB
            sel = identr[:, k_in_blk * BL:(k_in_blk + 1) * BL]
            # gates = xproj_rows + h @ whhT  (into ps_rec[0:BL, :])
            for n in range(NN):
                ncols = slice(n * 512, (n + 1) * 512)
                nc.tensor.matmul(
                    out=ps_rec[:BL, ncols],
                    lhsT=sel,
                    rhs=xp[:, ncols],
                    start=True,
                    stop=False,
                )
                for k in range(NK_H):
                    nc.tensor.matmul(
                        out=ps_rec[:BL, ncols],
                        lhsT=hT[:, k * BL:(k + 1) * BL],
                        rhs=whh_sb[:, k * G + n * 512:k * G + (n + 1) * 512],
                        start=False,
                        stop=(k == NK_H - 1),
                    )
            AF = mybir.ActivationFunctionType
            f_h = act_pool.tile([BL, 512], F32, tag="fh")
            g_h = act_pool.tile([BL, 512], F32, tag="gh")
            i_h = act_pool.tile([BL, 512], F32, tag="ih")
            o_h = act_pool.tile([BL, 512], F32, tag="oh")
            nc.scalar.activation(f_h[:], ps_rec[:BL, BLK_F * 512:(BLK_F + 1) * 512], AF.Sigmoid)
            nc.scalar.activation(g_h[:], ps_rec[:BL, BLK_G * 512:(BLK_G + 1) * 512], AF.Tanh)
            nc.scalar.activation(i_h[:], ps_rec[:BL, BLK_I * 512:(BLK_I + 1) * 512], AF.Sigmoid)
            nc.scalar.activation(o_h[:], ps_rec[:BL, BLK_O * 512:(BLK_O + 1) * 512], AF.Sigmoid)

            m1 = tmp_pool.tile([BL, 512], F32, tag="m1")
            m2 = tmp_pool.tile([BL, 512], F32, tag="m2")
            nc.vector.tensor_mul(m1[:], f_h[:], c_st[:])
            nc.vector.tensor_mul(m2[:], i_h[:], g_h[:])
            nc.vector.tensor_add(c_st[:], m1[:], m2[:])
            tc_h = tmp_pool.tile([BL, 512], F32, tag="tc")
            nc.scalar.activation(tc_h[:], c_st[:], AF.Tanh)
            h_new = hr_pool.tile([BL, 512], F32, tag="h")
            nc.vector.tensor_mul(h_new[:], o_h[:], tc_h[:])

            # h^T for next step's stationary: 4 PE transposes into ps_rec[:, :64]
            for k in range(NK_H):
                nc.tensor.transpose(
                    out=ps_rec[:, k * BL:(k + 1) * BL],
                    in_=h_new[:, k * 128:(k + 1) * 128],
                    identity=ident[:BL, :BL],
                )
            nc.vector.tensor_copy(hT[:], ps_rec[:, :NK_H * BL])

            # scatter this step's output rows
            nc.gpsimd.indirect_dma_start(
                out=out_d[:],
                out_offset=bass.IndirectOffsetOnAxis(
                    ap=soff_sb[:, s:s + 1], axis=0
                ),
                in_=h_new[:],
                in_offset=None,
            )

        xp_cur = background(0)
        for c in range(nchunks):
            if c + 1 < nchunks:
                xp_next = background(c + 1)
            else:
                xp_next = None
            for l in range(CT):
                s = c * CT + l
                step(s, xp_cur[l // SPB])
            xp_cur = xp_next

    nc.compile()
    return nc


def make_core_inputs(inputs, core, t_steps=T):
    """Build the in_map for one core from the full problem inputs."""
    fwd = core < NCORES // 2
    sl = slice((core % 4) * BL, (core % 4) * BL + BL)
    x = np.asarray(inputs["input"], np.float32)[:t_steps, sl, :]
    lengths = np.asarray(inputs["lengths"]).astype(np.int64)[sl]
    sfx = "f" if fwd else "b"
    w_ih = np.asarray(inputs[f"w_ih_{sfx}"], np.float32)
    w_hh = np.asarray(inputs[f"w_hh_{sfx}"], np.float32)
    bias = (
        np.asarray(inputs[f"b_ih_{sfx}"], np.float32)
        + np.asarray(inputs[f"b_hh_{sfx}"], np.float32)
    )
    h0 = np.asarray(inputs[f"h0_{sfx}"], np.float32)[sl]
    c0 = np.asarray(inputs[f"c0_{sfx}"], np.float32)[sl]

    t = np.arange(t_steps, dtype=np.int64)[:, None]
    if fwd:
        idx = np.broadcast_to(t, (t_steps, BL))
    else:
        L = lengths[None, :]
        idx = np.where(t < L, L - 1 - t, t)
    offs = (idx * BL + np.arange(BL)[None, :]).astype(np.int32)  # [T, BL]
    goff = offs.reshape(-1, RPB).T.copy()       # [128, nblocks]
    soff = offs.T.copy()                        # [BL, T]

    return {
        "x": np.ascontiguousarray(x.reshape(t_steps * BL, I)),
        "wihT": np.ascontiguousarray(w_ih.T[:, _PERM]),
        "whhT": np.ascontiguousarray(w_hh.T[:, _PERM]),
        "bias": np.ascontiguousarray(bias[_PERM][None, :]),
        "h0T": np.ascontiguousarray(h0.T),
        "c0": np.ascontiguousarray(c0),
        "goff": np.ascontiguousarray(goff),
        "soff": np.ascontiguousarray(soff),
    }


_PROGRAM_CACHE = {}


def kernel(**inputs) -> np.ndarray:
    t_steps = inputs["input"].shape[0]
    if t_steps not in _PROGRAM_CACHE:
        _PROGRAM_CACHE[t_steps] = build_program(t_steps)
    nc = _PROGRAM_CACHE[t_steps]
    in_maps = [make_core_inputs(inputs, c, t_steps) for c in range(NCORES)]
    res = run_bass_kernel_spmd(nc, in_maps, list(range(NCORES)))
    out = np.empty((t_steps, B, 2 * H), np.float32)
    for c in range(NCORES):
        sl = slice((c % 4) * BL, (c % 4) * BL + BL)
        half = slice(0, H) if c < 4 else slice(H, 2 * H)
        out[:, sl, half] = res.results[c]["out"].reshape(t_steps, BL, H)
    return out


if __name__ == "__main__":
    pass


# revision 5
# speedup vs baseline: 1.9004x; 1.2338x over previous
"""Bidirectional LSTM layer (T=1024, B=64, I=H=512) on 8 Trainium2 NeuronCores.

Sharding: cores 0-3 run the forward direction, cores 4-7 the backward
direction; each core owns 16 batch columns and runs the full sequential scan.
The length-dependent flip (flip_batch) is done on-device with indirect DMA
gather/scatter driven by small host-computed index tables.
"""

import sys
from contextlib import ExitStack

import numpy as np

for p in ("/opt/trn_rl_repo", "/root/.axon_site/_ro/trn_rl_repo"):
    if p not in sys.path:
        sys.path.append(p)

import concourse.bass as bass
import concourse.tile as tile
from concourse import bacc, mybir
from concourse.bass_utils import run_bass_kernel_spmd
from concourse.masks import make_identity

F32 = mybir.dt.float32
F32R = mybir.dt.float32r
I32 = mybir.dt.int32


def r32(ap):
    return ap.bitcast(F32R)

T, B, I, H = 1024, 64, 512, 512
G = 4 * H            # gate width
NCORES = 8
BL = B // (NCORES // 2)   # 16 batch columns per core
CT = 16                   # timesteps per pipeline chunk
RPB = 128                 # rows per projection block
SPB = RPB // BL           # steps per projection block (8)
BPC = CT // SPB           # projection blocks per chunk (2)
NK_I = I // 128           # k-tiles for input dim (4)
NK_H = H // 128           # k-tiles for hidden dim (4)
NN = G // 512             # 512-wide gate column chunks (4)

# gate column order in the reordered weights: (i, g, f, o)
# torch row order is (i, f, g, o)
_PERM = np.concatenate([
    np.arange(0, H),          # i
    np.arange(2 * H, 3 * H),  # g
    np.arange(H, 2 * H),      # f
    np.arange(3 * H, 4 * H),  # o
])
BLK_I, BLK_G, BLK_F, BLK_O = 0, 1, 2, 3


def build_program(t_steps=T):
    nsteps = t_steps
    nchunks = nsteps // CT
    nc = bacc.Bacc("TRN2", target_bir_lowering=False, debug=False)

    # DRAM I/O (per-core values supplied via in_maps)
    x_d = nc.dram_tensor("x", [nsteps * BL, I], F32, kind="ExternalInput").ap()
    wih_d = nc.dram_tensor("wihT", [I, G], F32R, kind="ExternalInput").ap()
    whh_d = nc.dram_tensor("whhT", [H, G], F32R, kind="ExternalInput").ap()
    bias_d = nc.dram_tensor("bias", [1, G], F32R, kind="ExternalInput").ap()
    h0T_d = nc.dram_tensor("h0T", [H, BL], F32R, kind="ExternalInput").ap()
    c0_d = nc.dram_tensor("c0", [BL, H], F32, kind="ExternalInput").ap()
    # gather offsets: [128, nblocks] column b = row offsets of projection block b
    goff_d = nc.dram_tensor(
        "goff", [RPB, nsteps * BL // RPB], I32, kind="ExternalInput"
    ).ap()
    # scatter offsets: [BL, nsteps]
    soff_d = nc.dram_tensor("soff", [BL, nsteps], I32, kind="ExternalInput").ap()
    out_d = nc.dram_tensor("out", [nsteps * BL, H], F32, kind="ExternalOutput").ap()

    with tile.TileContext(nc) as tc, ExitStack() as ctx:
        cpool = ctx.enter_context(tc.tile_pool(name="consts", bufs=1))
        ident = cpool.tile([128, 128], F32, tag="ident")
        make_identity(nc, ident[:])
        identr = cpool.tile([128, 128], F32R, tag="identr")
        nc.vector.tensor_copy(identr[:], ident[:])
        ones_f = cpool.tile([1, 128], F32, tag="onesf")
        nc.vector.memset(ones_f[:], 1.0)
        ones_r = cpool.tile([1, 128], F32R, tag="ones")
        nc.vector.tensor_copy(ones_r[:], ones_f[:])

        wih_sb = cpool.tile([128, NK_I * G], F32R, tag="wih")
        whh_sb = cpool.tile([128, NK_H * G], F32R, tag="whh")
        for k in range(NK_I):
            nc.sync.dma_start(
                wih_sb[:, k * G:(k + 1) * G], wih_d[k * 128:(k + 1) * 128, :]
            )
        for k in range(NK_H):
            nc.sync.dma_start(
                whh_sb[:, k * G:(k + 1) * G], whh_d[k * 128:(k + 1) * 128, :]
            )
        bias_sb = cpool.tile([1, G], F32R, tag="bias")
        nc.sync.dma_start(bias_sb[:], bias_d[:])
        goff_sb = cpool.tile([RPB, nsteps * BL // RPB], I32, tag="goff")
        nc.sync.dma_start(goff_sb[:], goff_d[:])
        soff_sb = cpool.tile([BL, nsteps], I32, tag="soff")
        nc.sync.dma_start(soff_sb[:], soff_d[:])

        # live state
        hT = cpool.tile([128, NK_H * BL], F32R, tag="hT")      # h^T k-tiles
        c_st = cpool.tile([BL, H], F32, tag="c")
        for k in range(NK_H):
            nc.sync.dma_start(
                hT[:, k * BL:(k + 1) * BL], h0T_d[k * 128:(k + 1) * 128, :]
            )
        nc.sync.dma_start(c_st[:], c0_d[:])

        # pipeline pools
        xg_pool = ctx.enter_context(tc.tile_pool(name="xg", bufs=2 * BPC))
        xT_pool = ctx.enter_context(tc.tile_pool(name="xT", bufs=2))
        xp_pool = ctx.enter_context(tc.tile_pool(name="xproj", bufs=2 * BPC))
        act_pool = ctx.enter_context(tc.tile_pool(name="acts", bufs=2))
        tmp_pool = ctx.enter_context(tc.tile_pool(name="tmps", bufs=2))
        hr_pool = ctx.enter_context(tc.tile_pool(name="hring", bufs=4))
        ps_rec_pool = ctx.enter_context(
            tc.tile_pool(name="psrec", bufs=1, space="PSUM")
        )
        ps_bg_pool = ctx.enter_context(
            tc.tile_pool(name="psbg", bufs=1, space="PSUM")
        )
        ps_rec = ps_rec_pool.tile([128, 512 * NN], F32, tag="psrec")
        ps_bg = ps_bg_pool.tile([128, 512 * NN], F32, tag="psbg")

        def background(c):
            """Gather + transpose + project chunk c; returns xproj block tiles."""
            xp_tiles = []
            for b in range(BPC):
                blk = c * BPC + b
                xg = xg_pool.tile([RPB, I], F32, tag="xg")
                nc.gpsimd.indirect_dma_start(
                    out=xg[:],
                    out_offset=None,
                    in_=x_d[:],
                    in_offset=bass.IndirectOffsetOnAxis(
                        ap=goff_sb[:, blk:blk + 1], axis=0
                    ),
                )
                # transpose rows->features: xg [128r, 512i] -> xT 4x[128i, 128r]
                xT = xT_pool.tile([128, I], F32R, tag="xT")
                for k in range(NK_I):
                    nc.tensor.transpose(
                        out=ps_bg[:, k * 128:(k + 1) * 128],
                        in_=xg[:, k * 128:(k + 1) * 128],
                        identity=ident[:],
                    )
                nc.vector.tensor_copy(xT[:], ps_bg[:, :I])
                # projection GEMM: xproj[128r, G] = xT.T @ wihT + bias
                xp = xp_pool.tile([RPB, G], F32R, tag="xp")
                for n in range(NN):
                    ncols = slice(n * 512, (n + 1) * 512)
                    for k in range(NK_I):
                        nc.tensor.matmul(
                            out=ps_bg[:, ncols],
                            lhsT=xT[:, k * 128:(k + 1) * 128],
                            rhs=wih_sb[:, k * G + n * 512:k * G + (n + 1) * 512],
                            start=(k == 0),
                            stop=False,
                        )
                    nc.tensor.matmul(
                        out=ps_bg[:, ncols],
                        lhsT=ones_r[:1, :RPB],
                        rhs=bias_sb[:1, ncols],
                        start=False,
                        stop=True,
                    )
                nc.vector.tensor_copy(xp[:, : G // 2], ps_bg[:, : G // 2])
                nc.scalar.copy(xp[:, G // 2:], ps_bg[:, G // 2:])
                xp_tiles.append(xp)
            return xp_tiles

        def step(s, xp):
            """One LSTM timestep. xp: xproj block tile holding this step."""
            k_in_blk = (s % CT) % SPB
            sel = identr[:, k_in_blk * BL:(k_in_blk + 1) * BL]
            # gates = xproj_rows + h @ whhT  (into ps_rec[0:BL, :])
            for n in range(NN):
                ncols = slice(n * 512, (n + 1) * 512)
                nc.tensor.matmul(
                    out=ps_rec[:BL, ncols],
                    lhsT=sel,
                    rhs=xp[:, ncols],
                    start=True,
                    stop=False,
                )
                for k in range(NK_H):
                    nc.tensor.matmul(
                        out=ps_rec[:BL, ncols],
                        lhsT=hT[:, k * BL:(k + 1) * BL],
                        rhs=whh_sb[:, k * G + n * 512:k * G + (n + 1) * 512],
                        start=False,
                        stop=(k == NK_H - 1),
                    )
            AF = mybir.ActivationFunctionType
            i_h = act_pool.tile([BL, 512], F32, tag="ih")
            g_h = act_pool.tile([BL, 512], F32, tag="gh")
            f_h = act_pool.tile([BL, 512], F32, tag="fh")
            o_h = act_pool.tile([BL, 512], F32, tag="oh")
            nc.scalar.activation(i_h[:], ps_rec[:BL, BLK_I * 512:(BLK_I + 1) * 512], AF.Sigmoid)
            nc.scalar.activation(g_h[:], ps_rec[:BL, BLK_G * 512:(BLK_G + 1) * 512], AF.Tanh)
            nc.scalar.activation(f_h[:], ps_rec[:BL, BLK_F * 512:(BLK_F + 1) * 512], AF.Sigmoid)
            nc.scalar.activation(o_h[:], ps_rec[:BL, BLK_O * 512:(BLK_O + 1) * 512], AF.Sigmoid)

            # m2 = i*g on DVE (overlaps remaining matmuls); m1 = f*c on GpSimd
            m1 = tmp_pool.tile([BL, 512], F32, tag="m1")
            m2 = tmp_pool.tile([BL, 512], F32, tag="m2")
            HH = 256
            for hh in range(2):
                hs = slice(hh * HH, (hh + 1) * HH)
                nc.vector.tensor_mul(m2[:, hs], i_h[:, hs], g_h[:, hs])
            for hh in range(2):
                hs = slice(hh * HH, (hh + 1) * HH)
                nc.gpsimd.tensor_mul(m1[:, hs], f_h[:, hs], c_st[:, hs])
            tc_h = tmp_pool.tile([BL, 512], F32, tag="tc")
            h_new = hr_pool.tile([BL, 512], F32, tag="h")
            # halves pipelined: c' -> tanh(c') -> h' ; transposes follow per half
            for hh in range(2):
                hs = slice(hh * HH, (hh + 1) * HH)
                nc.vector.tensor_add(c_st[:, hs], m1[:, hs], m2[:, hs])
                nc.scalar.activation(tc_h[:, hs], c_st[:, hs], AF.Tanh)
                nc.vector.tensor_mul(h_new[:, hs], o_h[:, hs], tc_h[:, hs])
                for k in (2 * hh, 2 * hh + 1):
                    nc.tensor.transpose(
                        out=ps_rec[:, k * BL:(k + 1) * BL],
                        in_=h_new[:, k * 128:(k + 1) * 128],
                        identity=ident[:BL, :BL],
                    )
            nc.vector.tensor_copy(hT[:], ps_rec[:, :NK_H * BL])

            # scatter this step's output rows
            nc.gpsimd.indirect_dma_start(
                out=out_d[:],
                out_offset=bass.IndirectOffsetOnAxis(
                    ap=soff_sb[:, s:s + 1], axis=0
                ),
                in_=h_new[:],
                in_offset=None,
            )

        xp_cur = background(0)
        for c in range(nchunks):
            if c + 1 < nchunks:
                xp_next = background(c + 1)
            else:
                xp_next = None
            for l in range(CT):
                s = c * CT + l
                step(s, xp_cur[l // SPB])
            xp_cur = xp_next

    nc.compile()
    return nc


def make_core_inputs(inputs, core, t_steps=T):
    """Build the in_map for one core from the full problem inputs."""
    fwd = core < NCORES // 2
    sl = slice((core % 4) * BL, (core % 4) * BL + BL)
    x = np.asarray(inputs["input"], np.float32)[:t_steps, sl, :]
    lengths = np.asarray(inputs["lengths"]).astype(np.int64)[sl]
    sfx = "f" if fwd else "b"
    w_ih = np.asarray(inputs[f"w_ih_{sfx}"], np.float32)
    w_hh = np.asarray(inputs[f"w_hh_{sfx}"], np.float32)
    bias = (
        np.asarray(inputs[f"b_ih_{sfx}"], np.float32)
        + np.asarray(inputs[f"b_hh_{sfx}"], np.float32)
    )
    h0 = np.asarray(inputs[f"h0_{sfx}"], np.float32)[sl]
    c0 = np.asarray(inputs[f"c0_{sfx}"], np.float32)[sl]

    t = np.arange(t_steps, dtype=np.int64)[:, None]
    if fwd:
        idx = np.broadcast_to(t, (t_steps, BL))
    else:
        L = lengths[None, :]
        idx = np.where(t < L, L - 1 - t, t)
    offs = (idx * BL + np.arange(BL)[None, :]).astype(np.int32)  # [T, BL]
    goff = offs.reshape(-1, RPB).T.copy()       # [128, nblocks]
    soff = offs.T.copy()                        # [BL, T]

    return {
        "x": np.ascontiguousarray(x.reshape(t_steps * BL, I)),
        "wihT": np.ascontiguousarray(w_ih.T[:, _PERM]),
        "whhT": np.ascontiguousarray(w_hh.T[:, _PERM]),
        "bias": np.ascontiguousarray(bias[_PERM][None, :]),
        "h0T": np.ascontiguousarray(h0.T),
        "c0": np.ascontiguousarray(c0),
        "goff": np.ascontiguousarray(goff),
        "soff": np.ascontiguousarray(soff),
    }


_PROGRAM_CACHE = {}


def kernel(**inputs) -> np.ndarray:
    t_steps = inputs["input"].shape[0]
    if t_steps not in _PROGRAM_CACHE:
        _PROGRAM_CACHE[t_steps] = build_program(t_steps)
    nc = _PROGRAM_CACHE[t_steps]
    in_maps = [make_core_inputs(inputs, c, t_steps) for c in range(NCORES)]
    res = run_bass_kernel_spmd(nc, in_maps, list(range(NCORES)))
    out = np.empty((t_steps, B, 2 * H), np.float32)
    for c in range(NCORES):
        sl = slice((c % 4) * BL, (c % 4) * BL + BL)
        half = slice(0, H) if c < 4 else slice(H, 2 * H)
        out[:, sl, half] = res.results[c]["out"].reshape(t_steps, BL, H)
    return out


if __name__ == "__main__":
    pass
